# revision 24
# baseline (speedup 1.0000x reference)
"""Trainium2 Bass kernel for nn_AttentionPositionAlign.

Reference computation (per batch b):
    src = query @ Wq                    # [M, H]
    tgt = memory @ Wm                   # [N, H]
    aligns = relu(src[:,None,:] + tgt[None,:,:])   # [M, N, H]
    out = aligns.reshape(M, N*H) @ Wout # [M, 4]

Strategy: data-parallel over B across the 8 NeuronCores (B == 8). All
compute happens in "transposed land" (H on SBUF partitions, M on the free
dim) so the Bahdanau broadcast-add becomes a per-partition scalar bias
that fuses into a single elementwise pass — the [B,M,N,H] intermediate
(604 MB) is never materialized:

    srcT[h, m] = (Wq.T @ query.T)[h, m]         PSUM-accumulated matmuls
    tgtT[h, n] = (Wm.T @ memory.T)[h, n]        direct-orientation matmuls
    for each (hc, n) chunk c (N*H/128 = 144 of them):
        Rt = relu(srcT[hc] + tgtT[hc][:, n])    ONE fused op per chunk:
                                                DVE tensor_scalar(add,max)
                                                or ACT activation(Relu,bias)
        psum_out[32g+k, m] += Wout_c.T @ Rt     col-tiled (tile_position)
                                                matmuls, 4 concurrent PE
                                                column groups, 144-deep
                                                PSUM accumulation
    out[k, m] = sum_g psum_out[32g+k, m]        selector matmul, then host
                                                transposes [4, M] -> [M, 4]

The relu stream is the floor: DVE tensor_scalar [128,1024] bf16 runs
back-to-back at ~396ns (4x mode), ACT ACTIVATE at (FD+311)/1.2 ns from
PSUM; with a ~34/110 ACT/DVE split both engines are ~fully busy for
~45us.  Everything else is prologue/epilogue engineering around the
measured DMA behavior: aggregate input bandwidth is ~290GB/s and the
SDMA engines split it ~evenly across ACTIVE rings, so the two HWDGE
rings (sync, scalar) carry everything in strict need-order and the
SWDGE ring is left idle (any early bulk on it steals from the critical
chain).  Measured ring behavior: data starts flowing ~1.3us after the
~0.65us dma_start issue; per-ring ~145GB/s while both are loaded.

 - sync:   qT0, qT1, wqr            (the srcT chain; qT0 lands ~12us)
 - scalar: wq0, mT, wm0, woSel, wm1, wm23   (tgt chain + trailing bulk)
 - src projection is emitted first (its inputs land first), tgt after;
   hc+1's projections are emitted INSIDE chunks(hc) (n=8/10) so their
   late weight DMAs can't head-block the contraction stream, and the
   PSUM->SBUF copies at n=16.
 - 9+16 dummy matmuls warm the PE HAM clock gate; one wq0-gated
   heartbeat restarts PE activity mid-DMA-window so the MID idle window
   can't re-throttle before the projections.
 - Epilogue: the last chunk's relu is emitted as two mc-half ops and
   the TAIL chunks' mc1 contraction matmuls are deferred so po[0]
   closes early; cast0 on ACT ∥ cast1 on DVE, output DMAs issue from
   sync and scalar in parallel.
"""

import numpy as np

import concourse.bass as bass
import concourse.tile as tile
from concourse import bacc, mybir
from concourse.bass_utils import run_bass_kernel_spmd

B, M, N, H = 8, 1024, 36, 512
DQ, DM = 512, 2048
P = 128
HC = H // P          # 4 h-chunks
DQC = DQ // P        # 4
DMC = DM // P        # 16
MC = 2               # m-chunks for 512-wide PSUM banks
MF = M // MC         # 512
NCHUNK = N * HC      # 144 contraction chunks of 128

f32 = mybir.dt.float32
f32r = mybir.dt.float32r
bf16 = mybir.dt.bfloat16

# Knobs
R_DT = bf16          # dtype of the relu output / contraction rhs+lhsT
SRC_DT = bf16        # dtype of the srcT store / relu input
IN_DT = bf16         # dtype inputs are shipped in
N_ACT = 34           # chunks assigned to ACT (rest on DVE)
COL_TILE = 4         # concurrent PE column groups for the contraction
# mc-split pipelining is disabled: Tile's dependency tracking makes the
# srcT copy wait for ALL hc0 projections (incl. qT1's mc1 half) anyway,
# so half-ops only add per-op overhead (~260ns/chunk on the DVE pole).
SPLIT_K = 0
TAIL = 4             # trailing chunks whose mc1 matmuls are deferred

WOSEL_W = NCHUNK * 4 + 4   # wo | sel

_CACHE = {}


N_ACT_LAST = NCHUNK - 8  # confine ACT chunks to the first 136


def _is_act(c):
    # Bresenham spread over the first N_ACT_LAST chunks only, so the
    # final 8 chunks all land on the faster DVE and the slow ACT op
    # never gates the post-relu reduce chain.
    if c >= N_ACT_LAST:
        return False
    return (c + 1) * N_ACT // N_ACT_LAST > c * N_ACT // N_ACT_LAST


def _build():
    nc = bacc.Bacc(
        "TRN2",
        target_bir_lowering=False,
        debug=False,
        num_devices=B,
        enable_partition_id=False,
    )

    # critQ = wq[hc0] | qT-mc0 ; critM = mT | wm[hc0] — merged so the
    # small tensors ride the big transfers' 4-5KB descriptors (1KB
    # descriptors measured ~40GB/s vs ~145GB/s for 4KB).
    critQ = nc.dram_tensor(
        "critQ", [P, DQC * P + DQC * MF], IN_DT, kind="ExternalInput"
    ).ap()
    critM = nc.dram_tensor(
        "critM", [P, DMC * N + DMC * P], IN_DT, kind="ExternalInput"
    ).ap()
    qT1 = nc.dram_tensor("qT1", [P, DQC * MF], IN_DT, kind="ExternalInput").ap()
    wosel = nc.dram_tensor("wosel", [P, WOSEL_W], R_DT, kind="ExternalInput").ap()
    wm1 = nc.dram_tensor("wm1", [P, DMC * P], IN_DT, kind="ExternalInput").ap()
    wm23 = nc.dram_tensor("wm23", [P, 2 * DMC * P], IN_DT, kind="ExternalInput").ap()
    wqr = nc.dram_tensor("wqr", [P, 3 * DQC * P], IN_DT, kind="ExternalInput").ap()
    ping = nc.dram_tensor("ping", [P, 16], IN_DT, kind="ExternalInput").ap()
    out = nc.dram_tensor("out", [4, M], f32, kind="ExternalOutput").ap()

    with tile.TileContext(nc) as tc:
        with (
            tc.tile_pool(name="weights", bufs=1) as wpool,
            tc.tile_pool(name="acts", bufs=1) as apool,
            tc.tile_pool(name="rpool", bufs=26) as rpool,
            tc.tile_pool(name="ppool", bufs=2, space="PSUM") as ppool,
            tc.tile_pool(name="opool", bufs=1, space="PSUM") as opool,
        ):
            # --- SBUF tiles
            critQ_sb = wpool.tile([P, DQC * P + DQC * MF], IN_DT)
            critM_sb = wpool.tile([P, DMC * N + DMC * P], IN_DT)
            wosel_sb = wpool.tile([P, WOSEL_W], R_DT)
            wmr_sb = wpool.tile([P, 3, DMC, P], IN_DT)
            wqr_sb = wpool.tile([P, 3, DQC, P], IN_DT)
            qT1_sb = wpool.tile([P, DQC, MF], IN_DT)

            def wq_view(hc, dq):
                if hc == 0:
                    return critQ_sb[:, dq * P : (dq + 1) * P]
                return wqr_sb[:, hc - 1, dq, :]

            def qT_view(mc, dq):
                if mc == 0:
                    o = DQC * P + dq * MF
                    return critQ_sb[:, o : o + MF]
                return qT1_sb[:, dq, :]

            def mT_view(dm):
                return critM_sb[:, dm * N : (dm + 1) * N]

            def wm_view(hc, dm):
                if hc == 0:
                    o = DMC * N + dm * P
                    return critM_sb[:, o : o + P]
                return wmr_sb[:, hc - 1, dm, :]

            def wo_view(c):
                return wosel_sb[:, 4 * c : 4 * c + 4]

            sel_view = wosel_sb[:, NCHUNK * 4 :]

            # --- DMA: two HWDGE rings, strict need-order FIFO.  The
            # tiny ping rides the otherwise-idle SWDGE ring: it lands
            # ~2us before the big transfers and re-warms the PE (below).
            ping_sb = wpool.tile([P, 16], IN_DT)
            nc.gpsimd.dma_start(ping_sb[:], ping[:])
            nc.sync.dma_start(critQ_sb[:], critQ[:])
            nc.scalar.dma_start(critM_sb[:], critM[:])
            nc.sync.dma_start(qT1_sb[:], qT1[:])
            nc.scalar.dma_start(wosel_sb[:], wosel[:])
            nc.scalar.dma_start(wmr_sb[:, 0, :, :], wm1[:])
            nc.sync.dma_start(wqr_sb[:], wqr[:])
            nc.scalar.dma_start(wmr_sb[:, 1:, :, :], wm23[:])

            # --- PE warm-up: the HAM clock gate holds the PE at 1.2 GHz
            # until it has been busy ~3.4us; idle >3.4us re-throttles.
            po = [opool.tile([P, MF], f32, name=f"po{mc}") for mc in range(MC)]
            warm = opool.tile([P, MF], f32, name="warm")
            zw = wpool.tile([P, MF], R_DT)
            nc.vector.memset(zw[:], 0.0)
            for _ in range(9):
                nc.tensor.matmul(
                    warm[:], zw[:, :P], zw[:],
                    start=True, stop=True, skip_group_check=True,
                )
            for _ in range(16):
                nc.tensor.matmul(
                    warm[:, :P], zw[:, :P], zw[:, :P],
                    start=True, stop=True, skip_group_check=True,
                )
            # zero-init po (sets has_written on all 128 partitions so
            # col-group matmuls can accumulate)
            for mc in range(MC):
                nc.tensor.matmul(
                    po[mc][:], zw[:, :P], zw[:],
                    start=True, stop=False, skip_group_check=True,
                )
            # ping-gated heartbeat: reading ping_sb makes Tile hold this
            # until the ping DMA lands (~10.5us), restarting PE activity
            # so the HAM MID window can't re-throttle before the
            # projections
            nc.tensor.matmul(
                warm[:4, :P], ping_sb[:, 0:4], zw[:, :P],
                start=True, stop=True, skip_group_check=True,
            )
            for _ in range(6):
                nc.tensor.matmul(
                    warm[:, :P], zw[:, :P], zw[:, :P],
                    start=True, stop=True, skip_group_check=True,
                )

            srcT_sb = apool.tile([P, HC, M], SRC_DT)
            tgt_sb = apool.tile([P, HC, N], f32)

            def proj_tgt(hc):
                # tgtT[hc] directly: out[h, n] += wm[dm]^T @ mT[dm]
                pz = opool.tile([P, N], f32, tag="tproj")
                for dm in range(DMC):
                    nc.tensor.matmul(
                        pz[:],
                        wm_view(hc, dm),
                        mT_view(dm),
                        start=(dm == 0),
                        stop=(dm == DMC - 1),
                    )
                return pz

            def proj_src(hc, stagger=False):
                # stagger (hc0 only): pin the mc1 matmuls later in the
                # scheduler's model so they can't interleave with mc0's —
                # otherwise the srcT-mc0 copy's PE-counter wait ends up
                # counting mc1 matmuls and the whole mc-split pipeline
                # serializes on the qT1 DMA.
                ps = ppool.tile([P, M], f32, tag="proj")
                from contextlib import nullcontext

                for mc in range(MC):
                    hint = (
                        tc.tile_wait_until(0.0086)
                        if (stagger and mc == 1)
                        else nullcontext()
                    )
                    with hint:
                        for dq in range(DQC):
                            nc.tensor.matmul(
                                ps[:, mc * MF : (mc + 1) * MF],
                                wq_view(hc, dq),
                                qT_view(mc, dq),
                                start=(dq == 0),
                                stop=(dq == DQC - 1),
                            )
                return ps

            def tgt_copy(hc, pz):
                nc.vector.tensor_copy(tgt_sb[:, hc, :], pz[:])

            def src_copy(hc, ps):
                # (GPSIMD cannot access PSUM, so the srcT copy stays on ACT)
                nc.scalar.copy(srcT_sb[:, hc, :], ps[:])

            def src_copy_half(hc, ps, mc):
                nc.scalar.copy(
                    srcT_sb[:, hc, mc * MF : (mc + 1) * MF],
                    ps[:, mc * MF : (mc + 1) * MF],
                )

            def chunks(hc, ps, emits=(), split=0, tail=0):
                rs = {}
                deferred = []

                def ops(n, mclo, mchi):
                    c = hc * N + n
                    if n in rs:
                        r = rs[n]
                    else:
                        r = rpool.tile([P, M], R_DT)
                        rs[n] = r
                    bias = tgt_sb[:, hc, n : n + 1]
                    sl = slice(mclo * MF, mchi * MF)
                    if _is_act(c):
                        # late ACT chunks switch to the SBUF srcT copy so
                        # ps[hc] frees early — otherwise proj_src(hc+2)'s
                        # PSUM-buffer WAR stalls the PE (and then DVE via
                        # r-tile reuse) at every hc boundary
                        src = ps[:, sl] if n < N - 8 else srcT_sb[:, hc, sl]
                        nc.scalar.activation(
                            r[:, sl],
                            src,
                            mybir.ActivationFunctionType.Relu,
                            bias=bias,
                            scale=1.0,
                        )
                    else:
                        nc.vector.tensor_scalar(
                            r[:, sl],
                            srcT_sb[:, hc, sl],
                            bias,
                            0.0,
                            mybir.AluOpType.add,
                            mybir.AluOpType.max,
                        )
                    g = c % COL_TILE
                    for mc in range(mclo, mchi):
                        mm = (
                            po[mc][32 * g : 32 * g + 4, :],
                            wo_view(c),
                            r[:, mc * MF : (mc + 1) * MF],
                        )
                        kw = dict(
                            start=False,
                            stop=(c >= NCHUNK - COL_TILE),
                            tile_position=(0, 32 * g),
                            skip_group_check=True,
                        )
                        if mc == 1 and n >= N - tail:
                            deferred.append((mm, kw))
                        else:
                            nc.tensor.matmul(*mm, **kw)

                for n in range(N):
                    for at, fn in emits:
                        if n == at:
                            fn()
                    if n < split:
                        # mc0 half only — runs while the qT mc1 half is
                        # still in flight
                        ops(n, 0, 1)
                        if n == 2:
                            src_copy_half(hc, ps, 1)
                        if n == split - 1:
                            for n2 in range(split):
                                ops(n2, 1, MC)
                    elif tail and n == N - 1:
                        # split the final relu into mc halves so po[0]'s
                        # last matmul issues ~0.35us earlier
                        ops(n, 0, 1)
                        ops(n, 1, MC)
                    else:
                        ops(n, 0, MC)
                return deferred

            # --- software-pipelined emission.  hc0: src first (its DMAs
            # land first), then tgt.  hc+1's projections are emitted
            # INSIDE chunks(hc) with tile_wait_until hints: Tile's cost
            # model thinks the weight DMAs land ~5x earlier than they
            # really do, and without the hints the scheduler hoists the
            # hc1-3 projections (and their PSUM->SBUF copies) to the head
            # of the PE/DVE/ACT queues, where their sem waits on late
            # DMAs head-block the whole relu stream.  Hint values are in
            # the scheduler's model clock (~chunk stream position).
            ps0 = proj_src(0)
            pz0 = proj_tgt(0)
            tgt_copy(0, pz0)
            src_copy(0, ps0)

            # per-hc (src, tgt, cp) hints in model-us
            HINTS = {
                1: (15.0, 16.0, 17.0),
                2: (17.5, 23.5, 25.8),
                3: (30.3, 37.2, 39.4),
            }

            ps_cur = ps0
            deferred = None
            for hc in range(HC):
                if hc + 1 < HC:
                    state = {}
                    h_src, h_tgt, h_cp = HINTS[hc + 1]

                    def em_src(h=hc + 1, hint=h_src):
                        with tc.tile_wait_until(hint / 1000.0):
                            state["ps"] = proj_src(h)

                    def em_tgt(h=hc + 1, hint=h_tgt):
                        with tc.tile_wait_until(hint / 1000.0):
                            state["pz"] = proj_tgt(h)

                    def em_cp(h=hc + 1, hint=h_cp):
                        with tc.tile_wait_until(hint / 1000.0):
                            tgt_copy(h, state["pz"])
                            src_copy(h, state["ps"])

                    chunks(
                        hc, ps_cur,
                        emits=((8, em_src), (10, em_tgt), (16, em_cp)),
                        split=SPLIT_K if hc == 0 else 0,
                    )
                    ps_cur = state["ps"]
                else:
                    deferred = chunks(hc, ps_cur, tail=TAIL)

            # --- epilogue: po[0] is closed (its tail matmuls were NOT
            # deferred) — cast it on ACT while po[1]'s deferred mc1
            # matmuls run, then cast po[1] on DVE.  Output DMAs issue
            # from sync and scalar in parallel.
            out_sb = apool.tile([4, M], f32)
            pf0 = apool.tile([P, MF], R_DT, name="pf0")
            pf1 = apool.tile([P, MF], R_DT, name="pf1")

            nc.scalar.copy(pf0[:], po[0][:])          # ACT cast, po0 done
            for mm, kw in deferred:                   # po1 mc1 tail
                nc.tensor.matmul(*mm, **kw)
            ro0 = warm[:4, :]
            nc.tensor.matmul(
                ro0, sel_view, pf0[:],
                start=True, stop=True, skip_group_check=True,
            )
            nc.vector.tensor_copy(pf1[:], po[1][:])   # DVE cast
            ro1 = po[0][:4, :]
            nc.tensor.matmul(
                ro1, sel_view, pf1[:],
                start=True, stop=True, skip_group_check=True,
            )
            nc.vector.tensor_copy(out_sb[:, :MF], ro0)
            nc.sync.dma_start(out[:, :MF], out_sb[:, :MF], single_packet=True)
            nc.scalar.copy(out_sb[:, MF:], ro1)
            nc.scalar.dma_start(out[:, MF:], out_sb[:, MF:], single_packet=True)

    nc.compile()
    return nc


def _sel_array():
    s = np.zeros((P, 4), np.float32)
    for p in range(P):
        if p % 32 < 4:
            s[p, p % 32] = 1.0
    return s


def _np_bf16():
    import ml_dtypes

    return ml_dtypes.bfloat16


def kernel(query, memory, Wq, Wm, Wout):
    if "nc" not in _CACHE:
        _CACHE["nc"] = _build()
    nc = _CACHE["nc"]
    in_maps = _make_in_maps(query, memory, Wq, Wm, Wout)
    res = run_bass_kernel_spmd(nc, in_maps, list(range(B)))
    return np.stack([res.results[b]["out"].T for b in range(B)]).astype(np.float32)


def _make_in_maps(query, memory, Wq, Wm, Wout):
    bf = _np_bf16()
    # wq packed [p, (hc, dq, 128)]: Wq[dq*128+p, hc*128+j]
    wq_p = np.ascontiguousarray(
        np.asarray(Wq, np.float32).reshape(DQC, P, HC, P).transpose(1, 2, 0, 3)
    ).reshape(P, HC, DQC * P)
    # wm packed [hi, (hc, dm, 128)]: Wm[dm*128+hi, hc*128+hin]
    wm_p = np.ascontiguousarray(
        np.asarray(Wm, np.float32).reshape(DMC, P, HC, P).transpose(1, 2, 0, 3)
    ).reshape(P, HC, DMC * P)
    # Wout rows are n*H + hc*128 + p; kernel chunk id c = hc*N + n (hc-major)
    wo_p = np.ascontiguousarray(
        np.asarray(Wout, np.float32).reshape(N, HC, P, 4).transpose(2, 1, 0, 3)
    ).reshape(P, NCHUNK * 4)
    wosel = np.concatenate([wo_p, _sel_array()], axis=1).astype(bf)
    wq0_p = np.ascontiguousarray(wq_p[:, 0, :])
    wqr_p = wq_p[:, 1:, :].reshape(P, 3 * DQC * P).astype(bf)
    wm0_p = np.ascontiguousarray(wm_p[:, 0, :])
    wm1_p = np.ascontiguousarray(wm_p[:, 1, :]).astype(bf)
    wm23_p = np.ascontiguousarray(wm_p[:, 2:, :].reshape(P, 2 * DMC * P)).astype(bf)
    in_maps = []
    for b in range(B):
        # qT packed [p, (mh, dq, 512)]: queryT[dq*128+p, mh*512+j]
        qm = (
            np.asarray(query[b], np.float32)
            .T.reshape(DQC, P, MC, MF)
            .transpose(1, 2, 0, 3)
        )
        qT0_p = np.ascontiguousarray(qm[:, 0]).reshape(P, DQC * MF)
        qT1_p = np.ascontiguousarray(qm[:, 1]).reshape(P, DQC * MF).astype(bf)
        mT_p = np.ascontiguousarray(
            np.asarray(memory[b], np.float32).T.reshape(DMC, P, N).transpose(1, 0, 2)
        ).reshape(P, DMC * N)
        m = {
            "critQ": np.concatenate([wq0_p, qT0_p], axis=1).astype(bf),
            "critM": np.concatenate([mT_p, wm0_p], axis=1).astype(bf),
            "qT1": qT1_p,
            "wosel": wosel,
            "wm1": wm1_p,
            "wm23": wm23_p,
            "wqr": wqr_p,
            "ping": np.zeros((P, 16), _np_bf16()),
        }
        in_maps.append(m)
    return in_maps


def bench(inputs, iters=20):
    """Time repeated executions of the compiled kernel with inputs resident
    on device. Returns a list of per-call wall seconds."""
    import time

    import jax
    from jax.sharding import Mesh, PartitionSpec
    from jax.experimental.shard_map import shard_map

    from concourse import bass2jax, mybir as _mybir

    if "nc" not in _CACHE:
        _CACHE["nc"] = _build()
    nc = _CACHE["nc"]
    in_maps = _make_in_maps(**inputs)

    bass2jax.install_neuronx_cc_hook()
    partition_name = nc.partition_id_tensor.name if nc.partition_id_tensor else None
    in_names, out_names, out_avals, zero_outs = [], [], [], []
    for alloc in nc.m.functions[0].allocations:
        if not isinstance(alloc, _mybir.MemoryLocationSet):
            continue
        name = alloc.memorylocations[0].name
        if alloc.kind == "ExternalInput":
            if name != partition_name:
                in_names.append(name)
        elif alloc.kind == "ExternalOutput":
            shape = tuple(alloc.tensor_shape)
            dtype = _mybir.dt.np(alloc.dtype)
            out_names.append(name)
            out_avals.append(jax.core.ShapedArray(shape, dtype))
            zero_outs.append(np.zeros(shape, dtype))
    n_params = len(in_names)
    n_outs = len(out_avals)
    all_in_names = list(in_names) + list(out_names)
    if partition_name is not None:
        all_in_names.append(partition_name)

    def _body(*args):
        operands = list(args)
        if partition_name is not None:
            operands.append(bass2jax.partition_id_tensor())
        outs = bass2jax._bass_exec_p.bind(
            *operands,
            out_avals=tuple(out_avals),
            in_names=tuple(all_in_names),
            out_names=tuple(out_names),
            lowering_input_output_aliases=(),
            sim_require_finite=True,
            sim_require_nnan=True,
            nc=nc,
        )
        return tuple(outs)

    devices = jax.devices()[:B]
    mesh = Mesh(np.asarray(devices), ("core",))
    in_specs = (PartitionSpec("core"),) * (n_params + n_outs)
    out_specs = (PartitionSpec("core"),) * n_outs
    sharded = jax.jit(
        shard_map(
            _body, mesh=mesh, in_specs=in_specs, out_specs=out_specs, check_rep=False
        ),
        donate_argnums=tuple(range(n_params, n_params + n_outs)),
        keep_unused=True,
    )
    concat_in = [
        np.concatenate([np.asarray(in_maps[c][nm]) for c in range(B)], axis=0)
        for nm in in_names
    ]
    dev_in = [jax.device_put(a) for a in concat_in]

    def zeros():
        return [np.zeros((B * z.shape[0], *z.shape[1:]), z.dtype) for z in zero_outs]

    # warmup (compile)
    out = sharded(*dev_in, *zeros())
    jax.block_until_ready(out)

    times = []
    for _ in range(iters):
        t0 = time.perf_counter()
        out = sharded(*dev_in, *zeros())
        jax.block_until_ready(out)
        times.append(time.perf_counter() - t0)
    return times


# revision 25
# speedup vs baseline: 1.0178x; 1.0178x over previous
"""Trainium2 Bass kernel for nn_AttentionPositionAlign.

Reference computation (per batch b):
    src = query @ Wq                    # [M, H]
    tgt = memory @ Wm                   # [N, H]
    aligns = relu(src[:,None,:] + tgt[None,:,:])   # [M, N, H]
    out = aligns.reshape(M, N*H) @ Wout # [M, 4]

Strategy: data-parallel over B across the 8 NeuronCores (B == 8). All
compute happens in "transposed land" (H on SBUF partitions, M on the free
dim) so the Bahdanau broadcast-add becomes a per-partition scalar bias
that fuses into a single elementwise pass — the [B,M,N,H] intermediate
(604 MB) is never materialized:

    srcT[h, m] = (Wq.T @ query.T)[h, m]         PSUM-accumulated matmuls
    tgtT[h, n] = (Wm.T @ memory.T)[h, n]        direct-orientation matmuls
    for each (hc, n) chunk c (N*H/128 = 144 of them):
        Rt = relu(srcT[hc] + tgtT[hc][:, n])    ONE fused op per chunk:
                                                DVE tensor_scalar(add,max)
                                                or ACT activation(Relu,bias)
        psum_out[32g+k, m] += Wout_c.T @ Rt     col-tiled (tile_position)
                                                matmuls, 4 concurrent PE
                                                column groups, 144-deep
                                                PSUM accumulation
    out[k, m] = sum_g psum_out[32g+k, m]        selector matmul, then host
                                                transposes [4, M] -> [M, 4]

The relu stream is the floor: DVE tensor_scalar [128,1024] bf16 runs
back-to-back at ~396ns (4x mode), ACT ACTIVATE at (FD+311)/1.2 ns from
PSUM; with a ~34/110 ACT/DVE split both engines are ~fully busy for
~45us.  Everything else is prologue/epilogue engineering around the
measured DMA behavior: aggregate input bandwidth is ~290GB/s and the
SDMA engines split it ~evenly across ACTIVE rings, so the two HWDGE
rings (sync, scalar) carry everything in strict need-order and the
SWDGE ring is left idle (any early bulk on it steals from the critical
chain).  Measured ring behavior: data starts flowing ~1.3us after the
~0.65us dma_start issue; per-ring ~145GB/s while both are loaded.

 - sync:   qT0, qT1, wqr            (the srcT chain; qT0 lands ~12us)
 - scalar: wq0, mT, wm0, woSel, wm1, wm23   (tgt chain + trailing bulk)
 - src projection is emitted first (its inputs land first), tgt after;
   hc+1's projections are emitted INSIDE chunks(hc) (n=8/10) so their
   late weight DMAs can't head-block the contraction stream, and the
   PSUM->SBUF copies at n=16.
 - 9+16 dummy matmuls warm the PE HAM clock gate; one wq0-gated
   heartbeat restarts PE activity mid-DMA-window so the MID idle window
   can't re-throttle before the projections.
 - Epilogue: the last chunk's relu is emitted as two mc-half ops and
   the TAIL chunks' mc1 contraction matmuls are deferred so po[0]
   closes early; cast0 on ACT ∥ cast1 on DVE, output DMAs issue from
   sync and scalar in parallel.
"""

import numpy as np

import concourse.bass as bass
import concourse.tile as tile
from concourse import bacc, mybir
from concourse.bass_utils import run_bass_kernel_spmd

B, M, N, H = 8, 1024, 36, 512
DQ, DM = 512, 2048
P = 128
HC = H // P          # 4 h-chunks
DQC = DQ // P        # 4
DMC = DM // P        # 16
MC = 2               # m-chunks for 512-wide PSUM banks
MF = M // MC         # 512
NCHUNK = N * HC      # 144 contraction chunks of 128

f32 = mybir.dt.float32
f32r = mybir.dt.float32r
bf16 = mybir.dt.bfloat16

# Knobs
R_DT = bf16          # dtype of the relu output / contraction rhs+lhsT
SRC_DT = bf16        # dtype of the srcT store / relu input
IN_DT = bf16         # dtype inputs are shipped in
N_ACT = 34           # chunks assigned to ACT (rest on DVE)
COL_TILE = 4         # concurrent PE column groups for the contraction
SPLIT_K = 6          # leading hc0 chunks emitted as per-mc half-ops
TAIL = 4             # trailing chunks whose mc1 matmuls are deferred

WOSEL_W = NCHUNK * 4 + 4   # wo | sel

_CACHE = {}


N_ACT_LAST = NCHUNK - 8  # confine ACT chunks to the first 136


def _is_act(c):
    # Bresenham spread over the first N_ACT_LAST chunks only, so the
    # final 8 chunks all land on the faster DVE and the slow ACT op
    # never gates the post-relu reduce chain.
    if c >= N_ACT_LAST:
        return False
    return (c + 1) * N_ACT // N_ACT_LAST > c * N_ACT // N_ACT_LAST


def _build():
    nc = bacc.Bacc(
        "TRN2",
        target_bir_lowering=False,
        debug=False,
        num_devices=B,
        enable_partition_id=False,
    )

    # critQ = wq[hc0] | qT-mc0 ; critM = mT | wm[hc0] — merged so the
    # small tensors ride the big transfers' 4-5KB descriptors (1KB
    # descriptors measured ~40GB/s vs ~145GB/s for 4KB).
    critQ = nc.dram_tensor(
        "critQ", [P, DQC * P + DQC * MF], IN_DT, kind="ExternalInput"
    ).ap()
    critM = nc.dram_tensor(
        "critM", [P, DMC * N + DMC * P], IN_DT, kind="ExternalInput"
    ).ap()
    qT1 = nc.dram_tensor("qT1", [P, DQC * MF], IN_DT, kind="ExternalInput").ap()
    wosel = nc.dram_tensor("wosel", [P, WOSEL_W], R_DT, kind="ExternalInput").ap()
    wm1 = nc.dram_tensor("wm1", [P, DMC * P], IN_DT, kind="ExternalInput").ap()
    wm23 = nc.dram_tensor("wm23", [P, 2 * DMC * P], IN_DT, kind="ExternalInput").ap()
    wqr = nc.dram_tensor("wqr", [P, 3 * DQC * P], IN_DT, kind="ExternalInput").ap()
    out = nc.dram_tensor("out", [4, M], f32, kind="ExternalOutput").ap()

    with tile.TileContext(nc) as tc:
        with (
            tc.tile_pool(name="weights", bufs=1) as wpool,
            tc.tile_pool(name="acts", bufs=1) as apool,
            tc.tile_pool(name="rpool", bufs=26) as rpool,
            tc.tile_pool(name="ppool", bufs=2, space="PSUM") as ppool,
            tc.tile_pool(name="opool", bufs=1, space="PSUM") as opool,
        ):
            # --- SBUF tiles
            critQ_sb = wpool.tile([P, DQC * P + DQC * MF], IN_DT)
            critM_sb = wpool.tile([P, DMC * N + DMC * P], IN_DT)
            wosel_sb = wpool.tile([P, WOSEL_W], R_DT)
            wmr_sb = wpool.tile([P, 3, DMC, P], IN_DT)
            wqr_sb = wpool.tile([P, 3, DQC, P], IN_DT)
            qT1_sb = wpool.tile([P, DQC, MF], IN_DT)

            def wq_view(hc, dq):
                if hc == 0:
                    return critQ_sb[:, dq * P : (dq + 1) * P]
                return wqr_sb[:, hc - 1, dq, :]

            def qT_view(mc, dq):
                if mc == 0:
                    o = DQC * P + dq * MF
                    return critQ_sb[:, o : o + MF]
                return qT1_sb[:, dq, :]

            def mT_view(dm):
                return critM_sb[:, dm * N : (dm + 1) * N]

            def wm_view(hc, dm):
                if hc == 0:
                    o = DMC * N + dm * P
                    return critM_sb[:, o : o + P]
                return wmr_sb[:, hc - 1, dm, :]

            def wo_view(c):
                return wosel_sb[:, 4 * c : 4 * c + 4]

            sel_view = wosel_sb[:, NCHUNK * 4 :]

            # --- DMA: two HWDGE rings, strict need-order FIFO.
            nc.sync.dma_start(critQ_sb[:], critQ[:])
            nc.scalar.dma_start(critM_sb[:], critM[:])
            nc.sync.dma_start(qT1_sb[:], qT1[:])
            nc.scalar.dma_start(wosel_sb[:], wosel[:])
            nc.scalar.dma_start(wmr_sb[:, 0, :, :], wm1[:])
            nc.sync.dma_start(wqr_sb[:], wqr[:])
            nc.scalar.dma_start(wmr_sb[:, 1:, :, :], wm23[:])

            # --- PE warm-up: the HAM clock gate holds the PE at 1.2 GHz
            # until it has been busy ~3.4us; idle >3.4us re-throttles.
            po = [opool.tile([P, MF], f32, name=f"po{mc}") for mc in range(MC)]
            warm = opool.tile([P, MF], f32, name="warm")
            zw = wpool.tile([P, MF], R_DT)
            nc.vector.memset(zw[:], 0.0)
            for _ in range(9):
                nc.tensor.matmul(
                    warm[:], zw[:, :P], zw[:],
                    start=True, stop=True, skip_group_check=True,
                )
            for _ in range(16):
                nc.tensor.matmul(
                    warm[:, :P], zw[:, :P], zw[:, :P],
                    start=True, stop=True, skip_group_check=True,
                )
            # zero-init po (sets has_written on all 128 partitions so
            # col-group matmuls can accumulate)
            for mc in range(MC):
                nc.tensor.matmul(
                    po[mc][:], zw[:, :P], zw[:],
                    start=True, stop=False, skip_group_check=True,
                )
            # critM-gated heartbeat: reading critM_sb makes Tile hold
            # this until that DMA lands, restarting PE activity so the
            # HAM MID window can't re-throttle before the projections
            nc.tensor.matmul(
                warm[:4, :P], critM_sb[:, 0:4], zw[:, :P],
                start=True, stop=True, skip_group_check=True,
            )
            for _ in range(6):
                nc.tensor.matmul(
                    warm[:, :P], zw[:, :P], zw[:, :P],
                    start=True, stop=True, skip_group_check=True,
                )

            srcT_sb = apool.tile([P, HC, M], SRC_DT)
            tgt_sb = apool.tile([P, HC, N], f32)

            def proj_tgt(hc):
                # tgtT[hc] directly: out[h, n] += wm[dm]^T @ mT[dm]
                pz = opool.tile([P, N], f32, tag="tproj")
                for dm in range(DMC):
                    nc.tensor.matmul(
                        pz[:],
                        wm_view(hc, dm),
                        mT_view(dm),
                        start=(dm == 0),
                        stop=(dm == DMC - 1),
                    )
                return pz

            def proj_src(hc):
                ps = ppool.tile([P, M], f32, tag="proj")
                for mc in range(MC):
                    for dq in range(DQC):
                        nc.tensor.matmul(
                            ps[:, mc * MF : (mc + 1) * MF],
                            wq_view(hc, dq),
                            qT_view(mc, dq),
                            start=(dq == 0),
                            stop=(dq == DQC - 1),
                        )
                return ps

            def tgt_copy(hc, pz):
                nc.vector.tensor_copy(tgt_sb[:, hc, :], pz[:])

            def src_copy(hc, ps):
                # (GPSIMD cannot access PSUM, so the srcT copy stays on ACT)
                nc.scalar.copy(srcT_sb[:, hc, :], ps[:])

            def src_copy_half(hc, ps, mc):
                nc.scalar.copy(
                    srcT_sb[:, hc, mc * MF : (mc + 1) * MF],
                    ps[:, mc * MF : (mc + 1) * MF],
                )

            def chunks(hc, ps, emits=(), split=0, tail=0):
                rs = {}
                deferred = []

                def ops(n, mclo, mchi):
                    c = hc * N + n
                    if n in rs:
                        r = rs[n]
                    else:
                        r = rpool.tile([P, M], R_DT)
                        rs[n] = r
                    bias = tgt_sb[:, hc, n : n + 1]
                    sl = slice(mclo * MF, mchi * MF)
                    if _is_act(c):
                        # late ACT chunks switch to the SBUF srcT copy so
                        # ps[hc] frees early — otherwise proj_src(hc+2)'s
                        # PSUM-buffer WAR stalls the PE (and then DVE via
                        # r-tile reuse) at every hc boundary
                        src = ps[:, sl] if n < N - 8 else srcT_sb[:, hc, sl]
                        nc.scalar.activation(
                            r[:, sl],
                            src,
                            mybir.ActivationFunctionType.Relu,
                            bias=bias,
                            scale=1.0,
                        )
                    else:
                        nc.vector.tensor_scalar(
                            r[:, sl],
                            srcT_sb[:, hc, sl],
                            bias,
                            0.0,
                            mybir.AluOpType.add,
                            mybir.AluOpType.max,
                        )
                    g = c % COL_TILE
                    for mc in range(mclo, mchi):
                        mm = (
                            po[mc][32 * g : 32 * g + 4, :],
                            wo_view(c),
                            r[:, mc * MF : (mc + 1) * MF],
                        )
                        kw = dict(
                            start=False,
                            stop=(c >= NCHUNK - COL_TILE),
                            tile_position=(0, 32 * g),
                            skip_group_check=True,
                        )
                        if mc == 1 and n >= N - tail:
                            deferred.append((mm, kw))
                        else:
                            nc.tensor.matmul(*mm, **kw)

                for n in range(N):
                    for at, fn in emits:
                        if n == at:
                            fn()
                    if n < split:
                        # mc0 half only — runs while the qT mc1 half is
                        # still in flight
                        ops(n, 0, 1)
                        if n == 2:
                            src_copy_half(hc, ps, 1)
                        if n == split - 1:
                            for n2 in range(split):
                                ops(n2, 1, MC)
                    elif tail and n == N - 1:
                        # split the final relu into mc halves so po[0]'s
                        # last matmul issues ~0.35us earlier
                        ops(n, 0, 1)
                        ops(n, 1, MC)
                    else:
                        ops(n, 0, MC)
                return deferred

            # --- software-pipelined emission.  hc0: src first (its DMAs
            # land first), then tgt.  hc+1's projections are emitted
            # INSIDE chunks(hc) with tile_wait_until hints: Tile's cost
            # model thinks the weight DMAs land ~5x earlier than they
            # really do, and without the hints the scheduler hoists the
            # hc1-3 projections (and their PSUM->SBUF copies) to the head
            # of the PE/DVE/ACT queues, where their sem waits on late
            # DMAs head-block the whole relu stream.  Hint values are in
            # the scheduler's model clock (~chunk stream position).
            ps0 = proj_src(0)
            pz0 = proj_tgt(0)
            tgt_copy(0, pz0)
            src_copy_half(0, ps0, 0)

            # per-hc (src, tgt, cp) hints in model-us
            HINTS = {
                1: (15.0, 16.0, 17.0),
                2: (17.5, 23.5, 25.8),
                3: (30.3, 37.2, 39.4),
            }

            ps_cur = ps0
            deferred = None
            for hc in range(HC):
                if hc + 1 < HC:
                    state = {}
                    h_src, h_tgt, h_cp = HINTS[hc + 1]

                    def em_src(h=hc + 1, hint=h_src):
                        with tc.tile_wait_until(hint / 1000.0):
                            state["ps"] = proj_src(h)

                    def em_tgt(h=hc + 1, hint=h_tgt):
                        with tc.tile_wait_until(hint / 1000.0):
                            state["pz"] = proj_tgt(h)

                    def em_cp(h=hc + 1, hint=h_cp):
                        with tc.tile_wait_until(hint / 1000.0):
                            tgt_copy(h, state["pz"])
                            src_copy(h, state["ps"])

                    chunks(
                        hc, ps_cur,
                        emits=((8, em_src), (10, em_tgt), (16, em_cp)),
                        split=SPLIT_K if hc == 0 else 0,
                    )
                    ps_cur = state["ps"]
                else:
                    deferred = chunks(hc, ps_cur, tail=TAIL)

            # --- epilogue: po[0] is closed (its tail matmuls were NOT
            # deferred) — cast it on ACT while po[1]'s deferred mc1
            # matmuls run, then cast po[1] on DVE.  Output DMAs issue
            # from sync and scalar in parallel.
            out_sb = apool.tile([4, M], f32)
            pf0 = apool.tile([P, MF], R_DT, name="pf0")
            pf1 = apool.tile([P, MF], R_DT, name="pf1")

            nc.scalar.copy(pf0[:], po[0][:])          # ACT cast, po0 done
            for mm, kw in deferred:                   # po1 mc1 tail
                nc.tensor.matmul(*mm, **kw)
            ro0 = warm[:4, :]
            nc.tensor.matmul(
                ro0, sel_view, pf0[:],
                start=True, stop=True, skip_group_check=True,
            )
            nc.vector.tensor_copy(pf1[:], po[1][:])   # DVE cast
            ro1 = po[0][:4, :]
            nc.tensor.matmul(
                ro1, sel_view, pf1[:],
                start=True, stop=True, skip_group_check=True,
            )
            nc.vector.tensor_copy(out_sb[:, :MF], ro0)
            nc.sync.dma_start(out[:, :MF], out_sb[:, :MF], single_packet=True)
            nc.scalar.copy(out_sb[:, MF:], ro1)
            nc.scalar.dma_start(out[:, MF:], out_sb[:, MF:], single_packet=True)

    nc.compile()
    return nc


def _sel_array():
    s = np.zeros((P, 4), np.float32)
    for p in range(P):
        if p % 32 < 4:
            s[p, p % 32] = 1.0
    return s


def _np_bf16():
    import ml_dtypes

    return ml_dtypes.bfloat16


def kernel(query, memory, Wq, Wm, Wout):
    if "nc" not in _CACHE:
        _CACHE["nc"] = _build()
    nc = _CACHE["nc"]
    in_maps = _make_in_maps(query, memory, Wq, Wm, Wout)
    res = run_bass_kernel_spmd(nc, in_maps, list(range(B)))
    return np.stack([res.results[b]["out"].T for b in range(B)]).astype(np.float32)


def _make_in_maps(query, memory, Wq, Wm, Wout):
    bf = _np_bf16()
    # wq packed [p, (hc, dq, 128)]: Wq[dq*128+p, hc*128+j]
    wq_p = np.ascontiguousarray(
        np.asarray(Wq, np.float32).reshape(DQC, P, HC, P).transpose(1, 2, 0, 3)
    ).reshape(P, HC, DQC * P)
    # wm packed [hi, (hc, dm, 128)]: Wm[dm*128+hi, hc*128+hin]
    wm_p = np.ascontiguousarray(
        np.asarray(Wm, np.float32).reshape(DMC, P, HC, P).transpose(1, 2, 0, 3)
    ).reshape(P, HC, DMC * P)
    # Wout rows are n*H + hc*128 + p; kernel chunk id c = hc*N + n (hc-major)
    wo_p = np.ascontiguousarray(
        np.asarray(Wout, np.float32).reshape(N, HC, P, 4).transpose(2, 1, 0, 3)
    ).reshape(P, NCHUNK * 4)
    wosel = np.concatenate([wo_p, _sel_array()], axis=1).astype(bf)
    wq0_p = np.ascontiguousarray(wq_p[:, 0, :])
    wqr_p = wq_p[:, 1:, :].reshape(P, 3 * DQC * P).astype(bf)
    wm0_p = np.ascontiguousarray(wm_p[:, 0, :])
    wm1_p = np.ascontiguousarray(wm_p[:, 1, :]).astype(bf)
    wm23_p = np.ascontiguousarray(wm_p[:, 2:, :].reshape(P, 2 * DMC * P)).astype(bf)
    in_maps = []
    for b in range(B):
        # qT packed [p, (mh, dq, 512)]: queryT[dq*128+p, mh*512+j]
        qm = (
            np.asarray(query[b], np.float32)
            .T.reshape(DQC, P, MC, MF)
            .transpose(1, 2, 0, 3)
        )
        qT0_p = np.ascontiguousarray(qm[:, 0]).reshape(P, DQC * MF)
        qT1_p = np.ascontiguousarray(qm[:, 1]).reshape(P, DQC * MF).astype(bf)
        mT_p = np.ascontiguousarray(
            np.asarray(memory[b], np.float32).T.reshape(DMC, P, N).transpose(1, 0, 2)
        ).reshape(P, DMC * N)
        m = {
            "critQ": np.concatenate([wq0_p, qT0_p], axis=1).astype(bf),
            "critM": np.concatenate([mT_p, wm0_p], axis=1).astype(bf),
            "qT1": qT1_p,
            "wosel": wosel,
            "wm1": wm1_p,
            "wm23": wm23_p,
            "wqr": wqr_p,
        }
        in_maps.append(m)
    return in_maps


def bench(inputs, iters=20):
    """Time repeated executions of the compiled kernel with inputs resident
    on device. Returns a list of per-call wall seconds."""
    import time

    import jax
    from jax.sharding import Mesh, PartitionSpec
    from jax.experimental.shard_map import shard_map

    from concourse import bass2jax, mybir as _mybir

    if "nc" not in _CACHE:
        _CACHE["nc"] = _build()
    nc = _CACHE["nc"]
    in_maps = _make_in_maps(**inputs)

    bass2jax.install_neuronx_cc_hook()
    partition_name = nc.partition_id_tensor.name if nc.partition_id_tensor else None
    in_names, out_names, out_avals, zero_outs = [], [], [], []
    for alloc in nc.m.functions[0].allocations:
        if not isinstance(alloc, _mybir.MemoryLocationSet):
            continue
        name = alloc.memorylocations[0].name
        if alloc.kind == "ExternalInput":
            if name != partition_name:
                in_names.append(name)
        elif alloc.kind == "ExternalOutput":
            shape = tuple(alloc.tensor_shape)
            dtype = _mybir.dt.np(alloc.dtype)
            out_names.append(name)
            out_avals.append(jax.core.ShapedArray(shape, dtype))
            zero_outs.append(np.zeros(shape, dtype))
    n_params = len(in_names)
    n_outs = len(out_avals)
    all_in_names = list(in_names) + list(out_names)
    if partition_name is not None:
        all_in_names.append(partition_name)

    def _body(*args):
        operands = list(args)
        if partition_name is not None:
            operands.append(bass2jax.partition_id_tensor())
        outs = bass2jax._bass_exec_p.bind(
            *operands,
            out_avals=tuple(out_avals),
            in_names=tuple(all_in_names),
            out_names=tuple(out_names),
            lowering_input_output_aliases=(),
            sim_require_finite=True,
            sim_require_nnan=True,
            nc=nc,
        )
        return tuple(outs)

    devices = jax.devices()[:B]
    mesh = Mesh(np.asarray(devices), ("core",))
    in_specs = (PartitionSpec("core"),) * (n_params + n_outs)
    out_specs = (PartitionSpec("core"),) * n_outs
    sharded = jax.jit(
        shard_map(
            _body, mesh=mesh, in_specs=in_specs, out_specs=out_specs, check_rep=False
        ),
        donate_argnums=tuple(range(n_params, n_params + n_outs)),
        keep_unused=True,
    )
    concat_in = [
        np.concatenate([np.asarray(in_maps[c][nm]) for c in range(B)], axis=0)
        for nm in in_names
    ]
    dev_in = [jax.device_put(a) for a in concat_in]

    def zeros():
        return [np.zeros((B * z.shape[0], *z.shape[1:]), z.dtype) for z in zero_outs]

    # warmup (compile)
    out = sharded(*dev_in, *zeros())
    jax.block_until_ready(out)

    times = []
    for _ in range(iters):
        t0 = time.perf_counter()
        out = sharded(*dev_in, *zeros())
        jax.block_until_ready(out)
        times.append(time.perf_counter() - t0)
    return times
